# revision 52
# baseline (speedup 1.0000x reference)
"""Trainium2 Bass kernel for nn_MHA_29008209117536.

MHA with a temporal-bias MLP:
  q = (Xq Wq) split-heads; k/v from Xk; scores = qk^T/8 + bias(T); softmax; out = (attn v) Wp

Design (v5) -- built for this toolchain's serial-LDWEIGHTS cost model
(--enable-ldw-opt is force-false and =true crashes walrus codegen, so
every matmul pays LDW(cols)/1.2GHz + N/2.4GHz + ~60ns; the PE is the
bottleneck at ~42us and scalar/DVE/GPSIMD all have slack):
  * Host folds the bias MLP to E = exp(C/log(e+T)) (bf16); on device
    exp(scores)*E replaces exp(scores+bias).  All PE inputs bf16.
  * Sharding: data-parallel over batch; core b handles batch b.
  * PE warm-up: 3 dummy matmuls bridge the ~7us runtime preamble to the
    first input chunk so the HAM clock gate ramps early.
  * Input DMAs: ONE queue in strict first-need order (two queues round-
    robin and double the critical-path latency); wq/wk are host-packed
    so per-m chunks are contiguous; xq/xk split per-kt so the first
    QT/KT matmuls start as soon as chunk 0 lands.
  * T-space dataflow: QT/KT = W^T X^T (contract d_in), V natural,
    ST = K Q^T row-tiled (heads 2m/2m+1 on PE rows 0:64/64:128 run
    concurrently), AV = [V|1]^T P with the softmax denominator riding
    in the ones column, per-pair normalization:
      otu (DVE av->SBUF) || sm2 (scalar sums row) -> rvb = bf16
      reciprocal_approx_fast (custom-dve direct) -> bc = ones x rvb
      K=1 matmuls -> two DVE mults; odd head partition-shifted by DMA.
  * PSUM (8 banks): st pool 2x[128,512] (shared tag with bc/out23) +
    proj pool 2 + av pair pool 2x[128,1024] (4).
"""

import numpy as np
import ml_dtypes

import concourse.bass as bass
import concourse.mybir as mybir
import concourse.tile as tile
from concourse import bacc
from concourse.bass_utils import run_bass_kernel_spmd

F32 = mybir.dt.float32
BF16 = mybir.dt.bfloat16
AF = mybir.ActivationFunctionType
ALU = mybir.AluOpType

B, S, D, H = 8, 512, 512, 8
DK = D // H          # 64
P = 128              # partitions
NT = S // P          # 4 tiles of 128 along any 512 dim
S2 = 2 * S
N_CORES = 8
N_WARMUP = 3


def build_nc(use_bias: bool):
    nc = bacc.Bacc("TRN2", target_bir_lowering=False, debug=False,
                   num_devices=N_CORES)

    xqT = nc.dram_tensor("xqT", [D, S], BF16, kind="ExternalInput").ap()
    xkT = nc.dram_tensor("xkT", [D, S], BF16, kind="ExternalInput").ap()
    ebf = nc.dram_tensor("ebf", [S, S], BF16, kind="ExternalInput").ap()
    wq = nc.dram_tensor("wq", [D, D], BF16, kind="ExternalInput").ap()
    wk = nc.dram_tensor("wk", [D, D], BF16, kind="ExternalInput").ap()
    wv = nc.dram_tensor("wv", [D, D], BF16, kind="ExternalInput").ap()
    wp = nc.dram_tensor("wp", [D, D], BF16, kind="ExternalInput").ap()
    out = nc.dram_tensor("out", [S, D], F32, kind="ExternalOutput").ap()

    # wq/wk arrive HOST-PACKED as [m, p, kt*128+c]; view [p, m, ktc].
    wq_t = wq.rearrange("(m p) kc -> p m kc", p=P)
    wk_t = wk.rearrange("(m p) kc -> p m kc", p=P)
    wv_t = wv.rearrange("(kt p) d -> p kt d", p=P)
    wp_t = wp.rearrange("(kt p) d -> p kt d", p=P)
    xqT_t = xqT.rearrange("(kt p) s -> p kt s", p=P)
    xkT_t = xkT.rearrange("(kt p) s -> p kt s", p=P)
    ebf_t = ebf.rearrange("(kt p) s -> p kt s", p=P)
    out_t = out.rearrange("(st p) d -> p st d", p=P)

    with tile.TileContext(nc) as tc:
        with (
            tc.tile_pool(name="const", bufs=1) as cpool,
            tc.tile_pool(name="work", bufs=2) as wpool,
            tc.tile_pool(name="pt", bufs=12) as ptpool,
            tc.tile_pool(name="pj", bufs=2, space="PSUM") as pjp,
            tc.tile_pool(name="st", bufs=2, space="PSUM") as stp,
            tc.tile_pool(name="av", bufs=2, space="PSUM") as avp,
        ):
            ones_bf = cpool.tile([P, S], BF16, tag="ones_bf")
            xq_sb = cpool.tile([P, NT, S], BF16, tag="xq")
            xk_sb = cpool.tile([P, NT, S], BF16, tag="xk")
            wq_sb = cpool.tile([P, NT, NT, P], BF16, tag="wq")
            wk_sb = cpool.tile([P, NT, NT, P], BF16, tag="wk")
            wv_sb = cpool.tile([P, NT, D], BF16, tag="wv")
            wp_sb = cpool.tile([P, NT, D], BF16, tag="wp")
            qt_sb = cpool.tile([P, NT, S], BF16, tag="qt")
            kt_sb = cpool.tile([P, NT, S], BF16, tag="kt")
            vb_sb = cpool.tile([P, NT, H * (DK + 1)], BF16, tag="vb")
            if use_bias:
                ebf_sb = cpool.tile([P, NT, S], BF16, tag="ebf")
            ones_bc = cpool.tile([1, DK], BF16, tag="ones_bc")
            pairs = [cpool.tile([P, S], BF16, tag=f"pair{t}", name=f"pair{t}")
                     for t in range(NT)]

            # ---- PE warm-up ----
            nc.vector.memset(ones_bf, 1.0)
            dum = stp.tile([P, S], F32, tag="st", name="dum")
            for i in range(N_WARMUP):
                nc.tensor.matmul(dum, ones_bf[:, 0:P], ones_bf,
                                 start=True, stop=True)

            # ---- input DMAs: ONE hardware queue, strict first-need order.
            # A single queue keeps both the issue order and the HBM service
            # order aligned with the consumption order (two queues round-
            # robin at packet level and double the critical-path latency).
            nc.sync.dma_start(out=wq_sb[:, 0], in_=wq_t[:, 0])
            nc.sync.dma_start(out=xq_sb[:, 0:2], in_=xqT_t[:, 0:2])
            nc.sync.dma_start(out=xq_sb[:, 2:NT], in_=xqT_t[:, 2:NT])
            nc.sync.dma_start(out=wk_sb[:, 0], in_=wk_t[:, 0])
            if use_bias:
                nc.sync.dma_start(out=ebf_sb[:, 0:2], in_=ebf_t[:, 0:2])
            nc.sync.dma_start(out=xk_sb[:, 0:2], in_=xkT_t[:, 0:2])
            nc.sync.dma_start(out=xk_sb[:, 2:NT], in_=xkT_t[:, 2:NT])
            nc.sync.dma_start(out=wv_sb, in_=wv_t)
            nc.sync.dma_start(out=wq_sb[:, 1:NT], in_=wq_t[:, 1:NT])
            nc.sync.dma_start(out=wk_sb[:, 1:NT], in_=wk_t[:, 1:NT])
            if use_bias:
                nc.sync.dma_start(out=ebf_sb[:, 2:NT], in_=ebf_t[:, 2:NT])
            nc.sync.dma_start(out=wp_sb, in_=wp_t)

            # ones columns of the [V | 1] blocks
            vb_heads = vb_sb.rearrange("p kt (h c) -> p kt h c", c=DK + 1)
            one_sb = cpool.tile([P, NT, H, 1], F32, tag="ones")
            nc.vector.memset(one_sb, 1.0)
            nc.vector.memset(ones_bc, 1.0)
            with nc.allow_low_precision(reason="exact small constants"):
                nc.vector.tensor_copy(out=vb_heads[:, :, :, DK:DK + 1],
                                      in_=one_sb)

            # ---- projections ----
            def emit_qt(m):
                ps = pjp.tile([P, S], F32, tag="pj", name=f"qtps{m}")
                for kt in range(NT):
                    nc.tensor.matmul(ps, wq_sb[:, m, kt, :], xq_sb[:, kt, :],
                                     start=(kt == 0), stop=(kt == NT - 1))
                with nc.allow_low_precision(reason="bf16 activations"):
                    nc.vector.tensor_copy(out=qt_sb[:, m, :], in_=ps)

            def emit_kt(m):
                ps = pjp.tile([P, S], F32, tag="pj", name=f"ktps{m}")
                for kt in range(NT):
                    nc.tensor.matmul(ps, wk_sb[:, m, kt, :], xk_sb[:, kt, :],
                                     start=(kt == 0), stop=(kt == NT - 1))
                with nc.allow_low_precision(reason="bf16 activations"):
                    nc.vector.tensor_copy(out=kt_sb[:, m, :], in_=ps)

            def emit_v(sv):
                ps = pjp.tile([P, S], F32, tag="pj", name=f"vps{sv}")
                for kt in range(NT):
                    nc.tensor.matmul(ps,
                                     xk_sb[:, kt, sv * P:(sv + 1) * P],
                                     wv_sb[:, kt, :],
                                     start=(kt == 0), stop=(kt == NT - 1))
                with nc.allow_low_precision(reason="bf16 activations"):
                    nc.vector.tensor_copy(
                        out=vb_heads[:, sv, :, 0:DK],
                        in_=ps.rearrange("p (h c) -> p h c", c=DK))

            # ---- ST j-step: both heads of the pair row-tiled concurrent ----
            pts = {h: [] for h in range(H)}

            def emit_stj(p, j, order):
                tiles = {}
                for h in order:
                    tiles[h] = stp.tile([P, S], F32, tag="st",
                                        name=f"st_{h}_{j}")
                    hp = (h % 2) * DK
                    nc.tensor.matmul(
                        tiles[h],
                        kt_sb[hp:hp + DK, p, j * P:(j + 1) * P],
                        qt_sb[hp:hp + DK, p, :],
                        start=True, stop=True)
                for h in order:
                    pt = ptpool.tile([P, S], BF16, tag="pt",
                                     name=f"pt_{h}_{j}")
                    nc.scalar.activation(out=pt, in_=tiles[h], func=AF.Exp,
                                         scale=1.0)
                    if use_bias:
                        with nc.allow_low_precision(reason="bf16 attn"):
                            nc.vector.tensor_mul(out=pt, in0=pt,
                                                 in1=ebf_sb[:, j, :])
                    pts[h].append(pt)

            av_tiles = {}

            def emit_av(p, h, kts):
                if p not in av_tiles:
                    av_tiles[p] = avp.tile([P, S2], F32, tag="av",
                                           name=f"av_{p}")
                av = av_tiles[p]
                c = (h % 2) * S
                for kt in kts:
                    nc.tensor.matmul(
                        av[0:DK + 1, c:c + S],
                        vb_sb[:, kt, h * (DK + 1):(h + 1) * (DK + 1)],
                        pts[h][kt],
                        start=(kt == 0), stop=(kt == NT - 1))

            def emit_norm(p):
                # heads A=2p (cols 0:S), B=2p+1 (cols S:S2) of av pair tile.
                av = av_tiles[p]
                otu = wpool.tile([DK, S2], F32, tag="otu", bufs=2,
                                 name=f"otu_{p}")
                nc.vector.tensor_copy(out=otu, in_=av[0:DK, :])
                sm2 = wpool.tile([1, S2], F32, tag="sm2", bufs=2,
                                 name=f"sm2_{p}")
                nc.scalar.activation(out=sm2, in_=av[DK:DK + 1, :],
                                     func=AF.Copy, bias=0.0)
                rvb = wpool.tile([1, S2], BF16, tag="rvb", bufs=2,
                                 name=f"rvb_{p}")
                from concourse.dve_ops import (RECIP_APPROX_FAST_CONSTS,
                                               RECIPROCAL_APPROX_FAST)
                _c = RECIP_APPROX_FAST_CONSTS
                with nc.allow_low_precision(reason="bf16 normalization"):
                    nc.vector._custom_dve(RECIPROCAL_APPROX_FAST, out=rvb,
                                          in0=sm2, s0=_c["s0"], s1=_c["s1"],
                                          imm2=_c["imm2"])
                # bc rides in the av pool (its slot alternates with the av
                # pair tiles; the st pool's rotation would deadlock against
                # the long-lived out23 accumulators)
                bc = avp.tile([DK, S2], F32, tag="av", name=f"bc_{p}")
                nc.tensor.matmul(bc[:, 0:S], ones_bc, rvb[:, 0:S],
                                 start=True, stop=True)
                nc.tensor.matmul(bc[:, S:S2], ones_bc, rvb[:, S:S2],
                                 start=True, stop=True)
                # odd head first: its result takes the partition-shift DMA;
                # the even head's direct write finalizes the pair tile.
                otn = wpool.tile([DK, S], BF16, tag="otn", bufs=2,
                                 name=f"otn_{p}")
                with nc.allow_low_precision(reason="bf16 activations"):
                    nc.vector.tensor_tensor(out=otn, in0=otu[:, S:S2],
                                            in1=bc[:, S:S2], op=ALU.mult)
                    nc.sync.dma_start(out=pairs[p][DK:P, :], in_=otn)
                    nc.vector.tensor_tensor(out=pairs[p][0:DK, :],
                                            in0=otu[:, 0:S],
                                            in1=bc[:, 0:S], op=ALU.mult)

            # pair 3's normalize is split per head so the odd head's chain
            # (recip -> bc -> mult -> shift DMA) overlaps the even head's
            # remaining exps/AV instead of serializing into the tail.
            bc3 = {}

            def emit_norm3(hh):
                av = av_tiles[3]
                c = S if hh == "B" else 0
                if hh == "B":
                    bc3[0] = avp.tile([DK, S2], F32, tag="av", name="bc_3")
                bc = bc3[0][:, c:c + S]
                otu = wpool.tile([DK, S], F32, tag=f"otu3{hh}", bufs=1,
                                 name=f"otu3{hh}")
                nc.vector.tensor_copy(out=otu, in_=av[0:DK, c:c + S])
                sm = wpool.tile([1, S], F32, tag=f"sm3{hh}", bufs=1,
                                name=f"sm3{hh}")
                nc.scalar.activation(out=sm, in_=av[DK:DK + 1, c:c + S],
                                     func=AF.Copy, bias=0.0)
                rvb = wpool.tile([1, S], BF16, tag=f"rvb3{hh}", bufs=1,
                                 name=f"rvb3{hh}")
                from concourse.dve_ops import (RECIP_APPROX_FAST_CONSTS,
                                               RECIPROCAL_APPROX_FAST)
                _c = RECIP_APPROX_FAST_CONSTS
                with nc.allow_low_precision(reason="bf16 normalization"):
                    nc.vector._custom_dve(RECIPROCAL_APPROX_FAST, out=rvb,
                                          in0=sm, s0=_c["s0"], s1=_c["s1"],
                                          imm2=_c["imm2"])
                nc.tensor.matmul(bc, ones_bc, rvb, start=True, stop=True)
                with nc.allow_low_precision(reason="bf16 activations"):
                    if hh == "B":
                        otn = wpool.tile([DK, S], BF16, tag="otn3", bufs=1,
                                         name="otn3")
                        nc.vector.tensor_tensor(out=otn, in0=otu,
                                                in1=bc, op=ALU.mult)
                        nc.sync.dma_start(out=pairs[3][DK:P, :], in_=otn)
                    else:
                        nc.vector.tensor_tensor(out=pairs[3][0:DK, :],
                                                in0=otu, in1=bc,
                                                op=ALU.mult)

            out01 = {}

            def emit_out01(ts):
                for t in ts:
                    for st_ in (0, 1):
                        if t == 0:
                            out01[st_] = pjp.tile([P, S], F32, tag="pj",
                                                  name=f"out0{st_}")
                        nc.tensor.matmul(out01[st_],
                                         pairs[t][:, st_ * P:(st_ + 1) * P],
                                         wp_sb[:, t, :],
                                         start=(t == 0), stop=(t == NT - 1))

            out23 = {}

            def emit_out23(ts):
                if 0 in ts:
                    out23[2] = stp.tile([P, S], F32, tag="st", name="out2")
                    out23[3] = stp.tile([P, S], F32, tag="st", name="out3")
                for t in ts:
                    for st_ in (2, 3):
                        nc.tensor.matmul(out23[st_],
                                         pairs[t][:, st_ * P:(st_ + 1) * P],
                                         wp_sb[:, t, :],
                                         start=(t == 0), stop=(t == NT - 1))

            # ---- schedule ----
            # Within each block the PE queue alternates ST j-steps with
            # filler matmuls (prev pair's AV, next pair's projections, OUT)
            # so the in-order queue never blocks on the exp/mult chain.
            emit_qt(0)
            emit_kt(0)
            # pair 0 (heads 0,1)
            emit_stj(0, 0, (0, 1))
            emit_v(0)
            emit_stj(0, 1, (0, 1))
            emit_v(1)
            emit_stj(0, 2, (0, 1))
            emit_v(2)
            emit_stj(0, 3, (0, 1))
            emit_v(3)
            emit_av(0, 0, (0, 1))
            emit_av(0, 1, (0, 1))
            emit_qt(1)
            emit_kt(1)
            emit_av(0, 0, (2, 3))
            emit_av(0, 1, (2, 3))
            # pair 1 (heads 2,3)
            emit_stj(1, 0, (2, 3))
            emit_stj(1, 1, (2, 3))
            emit_qt(2)
            emit_norm(0)
            emit_stj(1, 2, (2, 3))
            emit_kt(2)
            emit_stj(1, 3, (2, 3))
            emit_av(1, 2, (0, 1))
            emit_av(1, 3, (0, 1))
            emit_av(1, 2, (2, 3))
            emit_av(1, 3, (2, 3))
            # pair 2 (heads 4,5)
            emit_stj(2, 0, (4, 5))
            emit_stj(2, 1, (4, 5))
            emit_qt(3)
            emit_norm(1)
            emit_stj(2, 2, (4, 5))
            emit_kt(3)
            emit_stj(2, 3, (4, 5))
            emit_av(2, 4, (0, 1))
            emit_av(2, 5, (0, 1))
            emit_av(2, 4, (2, 3))
            emit_av(2, 5, (2, 3))
            # pair 3 (heads 6,7) -- odd head's tiles first so the final
            # normalize ends on the even head's direct write
            emit_stj(3, 0, (7, 6))
            emit_stj(3, 1, (7, 6))
            emit_out01((0, 1))
            emit_norm(2)
            emit_stj(3, 2, (7, 6))
            emit_out01((2,))
            emit_stj(3, 3, (7, 6))
            emit_av(3, 7, (0, 1))
            emit_av(3, 6, (0, 1))
            emit_av(3, 7, (2, 3))
            emit_norm3("B")
            emit_av(3, 6, (2, 3))
            # tail: out23 t0..t2 fills the PE while the final normalize's
            # DVE chain runs
            emit_out23((0, 1, 2))
            emit_norm3("A")
            emit_out01((3,))
            emit_out23((3,))

            # ---- stores: four drains split DVE/scalar, but only TWO output
            # DMAs (each ~2us completion latency -- merging halves the tail)
            osb = wpool.tile([P, NT, S], F32, tag="osb", bufs=1, name="osb")
            nc.vector.tensor_copy(out=osb[:, 0], in_=out01[0])
            nc.scalar.activation(out=osb[:, 1], in_=out01[1],
                                 func=AF.Copy, bias=0.0)
            nc.sync.dma_start(out=out_t[:, 0:2, :], in_=osb[:, 0:2])
            nc.vector.tensor_copy(out=osb[:, 2], in_=out23[2])
            nc.scalar.activation(out=osb[:, 3], in_=out23[3],
                                 func=AF.Copy, bias=0.0)
            nc.scalar.dma_start(out=out_t[:, 2:4, :], in_=osb[:, 2:4])

    nc.compile()
    return nc


_CACHE = {}


def _get_nc(use_bias: bool):
    if use_bias not in _CACHE:
        _CACHE[use_bias] = build_nc(use_bias)
    return _CACHE[use_bias]


def prepare(inputs: dict):
    q = np.asarray(inputs["query_input"], dtype=np.float32)
    k = np.asarray(inputs["key_input"], dtype=np.float32)
    t = np.asarray(inputs["batch_temporal_mat"], dtype=np.float32)
    Wq = np.asarray(inputs["Wq"], dtype=np.float32)
    Wk = np.asarray(inputs["Wk"], dtype=np.float32)
    Wv = np.asarray(inputs["Wv"], dtype=np.float32)
    Wp = np.asarray(inputs["Wp"], dtype=np.float32)
    Wt1 = np.asarray(inputs["Wt1"], dtype=np.float32)[0]
    Wt2 = np.asarray(inputs["Wt2"], dtype=np.float32)[:, 0]

    C = float(np.sum(Wt2 * np.where(Wt1 >= 0.0, Wt1, 0.2 * Wt1),
                     dtype=np.float64))
    use_bias = abs(C) > 1e-20

    BF = ml_dtypes.bfloat16

    def pack_w(w):
        # [din, dout] -> [m, p, kt*128+c] with element (m,p,kt,c) =
        # w[kt*128+p, m*128+c]: per-m chunks contiguous for the DMA.
        return np.ascontiguousarray(
            w.reshape(NT, P, NT, P).transpose(2, 1, 0, 3).reshape(D, D)
            .astype(BF))

    wq_b = pack_w(Wq * np.float32(0.125))
    wk_b = pack_w(Wk)
    wv_b = np.ascontiguousarray(Wv.astype(BF))
    wp_b = np.ascontiguousarray(Wp.astype(BF))

    # E = exp(bias) elementwise, [sk, sq] orientation, bf16
    binv = 1.0 / np.log(np.float32(np.e) + t)          # [B, S, S]
    ebf = np.exp(np.float32(C) * binv).astype(BF)

    nc = _get_nc(use_bias)

    in_maps = []
    for b in range(N_CORES):
        m = {
            "xqT": np.ascontiguousarray(q[b].T.astype(BF)),
            "xkT": np.ascontiguousarray(k[b].T.astype(BF)),
            "wq": wq_b,
            "wk": wk_b,
            "wv": wv_b,
            "wp": wp_b,
            "ebf": np.ascontiguousarray(ebf[b].T),
        }
        in_maps.append(m)
    return nc, in_maps


def kernel(**inputs) -> np.ndarray:
    nc, in_maps = prepare(inputs)
    res = run_bass_kernel_spmd(nc, in_maps, list(range(N_CORES)), trace=False)
    return np.stack([res.results[b]["out"] for b in range(N_CORES)], axis=0)
